# revision 2
# baseline (speedup 1.0000x reference)
"""Trainium2 Bass kernel v4: fp8e4m3 DoubleRow matmul + u8 score export.

Device (per core, 16384 tokens, K=512 centers, C=256):
  - x and centers quantized host-side to fp8e4m3 (TRN float8e4).
  - Per 128-token tile: ONE DoubleRow matmul (contraction 256 = 2x128
    packed pairs) -> PSUM fp32 raw scores s = x.c_k.  ~1.5x the bf16 PE
    rate.
  - PSUM drained to u8 (q = s*SCALE + 128) split across DVE and ACT
    (the two engines with PSUM read ports); 4-tile PSUM groups, 2 banks
    per engine per group.
  - DMA: x in 512KB chunks, scores out 1MB chunks (>=512B/partition
    lines) to stay near HBM line rate.

Host: dequantize, add -0.5||c||^2 bias, argmax, flag small-margin or
saturated tokens, rescore them exactly in fp32, gather y = centers[idx].
"""
from contextlib import ExitStack

import numpy as np
import ml_dtypes

import concourse.bass as bass
import concourse.bacc as bacc
import concourse.mybir as mybir
import concourse.tile as tile
import concourse.bass_utils as bass_utils

B, H, W, C = 32, 64, 64, 256
K = 512
N_CORES = 8
P = 128
NTOK = B * H * W // N_CORES  # 16384

F32 = mybir.dt.float32
U8 = mybir.dt.uint8
FP8 = mybir.dt.float8e4
FP8_NP = ml_dtypes.float8_e4m3  # matches TRN float8e4 (max 240)

SUPER = 16            # tiles per DMA supergroup (2048 tokens)
PSG = 4               # tiles per PSUM group (4 banks)

SCALE = 1.22          # u8 = round(s * SCALE) + 128
OFFSET = 128.0
# flag threshold in dequantized units: covers fp8e4m3-matmul err + u8
# rounding (empirical minimal safe delta ~2.5 on this data)
FIXUP_DELTA = 3.5

_NC_CACHE = {}


def _build(ntok: int, num_devices: int, repeat: int = 1):
    ntiles = ntok // P
    nsuper = ntiles // SUPER
    SL = SUPER * P

    nc = bacc.Bacc("TRN2", target_bir_lowering=False, debug=False,
                   num_devices=num_devices)
    xT_d = nc.dram_tensor("xT", [C, ntok], FP8, kind="ExternalInput").ap()
    cT_d = nc.dram_tensor("cT", [C, K], FP8, kind="ExternalInput").ap()
    sc_d = nc.dram_tensor("scores", [ntok, K], U8, kind="ExternalOutput").ap()

    xT_v = xT_d.rearrange("(h p) n -> p h n", h=2)
    cT_v = cT_d.rearrange("(h p) k -> p h k", h=2)
    sc_v = sc_d.rearrange("(a p) k -> p a k", p=P)

    with tile.TileContext(nc) as tc, ExitStack() as ctx:
        constp = ctx.enter_context(tc.tile_pool(name="const", bufs=1))
        xp = ctx.enter_context(tc.tile_pool(name="x", bufs=3))
        scp = ctx.enter_context(tc.tile_pool(name="sc", bufs=3))
        psump = ctx.enter_context(
            tc.tile_pool(name="psum", bufs=2, space="PSUM"))

        ct = constp.tile([P, 2, K], FP8, tag="ct")
        nc.sync.dma_start(ct[:], cT_v[:])
        off = constp.tile([P, 1], F32, tag="off")
        nc.vector.memset(off[:], OFFSET)

        for _ in range(repeat):
            for g in range(nsuper):
                xs = xp.tile([P, 2, SL], FP8, tag="xs")
                nc.sync.dma_start(xs[:], xT_v[:, :, bass.ts(g, SL)])
                sc8 = scp.tile([P, SUPER, K], U8, tag="sc8")

                for q in range(SUPER // PSG):
                    ps = psump.tile([P, PSG, K], F32, tag="ps")
                    for j in range(PSG):
                        t = q * PSG + j
                        nc.tensor.matmul(
                            ps[:, j, :], xs[:, :, bass.ts(t, P)], ct[:],
                            start=True, stop=True,
                            perf_mode=mybir.MatmulPerfMode.DoubleRow)
                    h = PSG // 2
                    nc.vector.tensor_scalar(
                        sc8[:, q * PSG:q * PSG + h, :], ps[:, 0:h, :],
                        SCALE, OFFSET,
                        op0=mybir.AluOpType.mult, op1=mybir.AluOpType.add)
                    nc.scalar.activation(
                        sc8[:, q * PSG + h:(q + 1) * PSG, :], ps[:, h:PSG, :],
                        mybir.ActivationFunctionType.Identity,
                        bias=off[:], scale=SCALE)

                nc.sync.dma_start(sc_v[:, bass.ts(g, SUPER), :], sc8[:])

    nc.compile()
    return nc


def _host_postprocess(flat32, centers, scores_u8, c_sq, delta=FIXUP_DELTA):
    sc = scores_u8.astype(np.float32)
    sc -= OFFSET
    sc *= (1.0 / SCALE)
    sc -= 0.5 * c_sq[None, :]
    idx = np.argmax(sc, axis=-1)
    n = sc.shape[0]
    ar = np.arange(n)
    m1 = sc[ar, idx]
    sat = scores_u8[ar, idx] >= 254
    sc[ar, idx] = -np.inf
    m2 = sc.max(axis=-1)
    flag = ((m1 - m2) < delta) | sat
    if flag.any():
        xf = flat32[flag]
        d = c_sq[None, :] - 2.0 * (xf @ centers.T)
        idx[flag] = d.argmin(-1)
    return idx


def kernel(x: np.ndarray, centers: np.ndarray):
    x = np.asarray(x)
    centers = np.ascontiguousarray(np.asarray(centers, dtype=np.float32))
    assert x.shape == (B, H, W, C) and centers.shape == (K, C)

    key = (NTOK, N_CORES)
    if key not in _NC_CACHE:
        _NC_CACHE[key] = _build(NTOK, N_CORES)
    nc = _NC_CACHE[key]

    cT = np.ascontiguousarray(centers.T).astype(FP8_NP)
    flat32 = np.ascontiguousarray(x, dtype=np.float32).reshape(N_CORES, NTOK, C)
    in_maps = []
    for c in range(N_CORES):
        xT = np.ascontiguousarray(flat32[c].T).astype(FP8_NP)
        in_maps.append({"xT": xT, "cT": cT})

    res = bass_utils.run_bass_kernel_spmd(nc, in_maps,
                                          core_ids=list(range(N_CORES)))

    c_sq = (centers * centers).sum(-1)
    idx = np.empty((N_CORES, NTOK), dtype=np.int64)
    for c in range(N_CORES):
        scores = res.results[c]["scores"]
        idx[c] = _host_postprocess(flat32[c], centers, scores, c_sq)

    y = centers[idx.reshape(-1)].reshape(B, H, W, C)
    return (x, y)
